# revision 40
# baseline (speedup 1.0000x reference)
"""Trainium2 Bass kernel for windowed attention with dynamic position bias.

Shapes (hardcoded): qkv [3, 2, 65536, 192], H=W=256, window 8x32 (N=256),
6 heads, head_dim 32. 512 windows total, data-parallel over 8 cores
(64 windows each; each core owns a contiguous band of 64 H-rows of one batch).

v8 design:
  - Q^T / K^T built on host (numpy) -> no PE transposes on device.
  - fp16 Q/K/V/P throughout the matmuls; fp32 PSUM accumulation.
  - Scores per half (3 heads) into one [128,1536] PSUM tile; the K=32
    score matmuls use distinct PE row groups (base partitions 0/32/64)
    and run concurrently. One exp ACTIVATE per half (the bottleneck
    engine, ~1.5us/call back-to-back).
  - Position bias applied post-exp as P = exp(scale*S) * exp(bias) with
    exp(bias) precomputed on host; the multiply runs on the Vector engine.
  - AV with ones-column denominator trick, software-pipelined two windows
    behind scores so its inputs are always ready.
  - No on-device softmax division: the [numerators | denominator] PSUM
    block is copied to SBUF and DMA'd out raw; the host does the divide.
"""
import sys
import numpy as np

sys.path.insert(0, "/opt/trn_rl_repo")

H_SP, W_SP = 8, 32
NUM_HEADS = 6
DIM = 192
HEAD_DIM = 32
N = H_SP * W_SP          # 256 tokens per window
LN_EPS = 1e-5
SCALE = HEAD_DIM ** -0.5
B, H, W = 2, 256, 256
L = H * W
N_CORES = 8
WINDOWS_PER_CORE = 64    # 8 hb bands x 8 wi
L_PER_CORE = L // 4      # 16384 tokens (64 H-rows)

_BUILT = None


def _np_layer_norm(x, g, b):
    m = x.mean(axis=-1, keepdims=True)
    v = ((x - m) ** 2).mean(axis=-1, keepdims=True)
    return (x - m) / np.sqrt(v + LN_EPS) * g + b


def _host_eb(rpi, rpe_biases, p):
    """DynamicPosBias MLP + gather -> exp(bias) [128, 3072] fp16.

    col = half*1536 + h_local*512 + kk*256 + q ; partition p = k - kk*128,
    head h = 3*half + h_local. Matches the device score-PSUM layout.
    """
    x = rpe_biases.astype(np.float32)
    pos = x @ p["pos_proj_w"].T + p["pos_proj_b"]
    pos = np.maximum(_np_layer_norm(pos, p["ln1_g"], p["ln1_b"]), 0.0) @ p["fc1_w"].T + p["fc1_b"]
    pos = np.maximum(_np_layer_norm(pos, p["ln2_g"], p["ln2_b"]), 0.0) @ p["fc2_w"].T + p["fc2_b"]
    pos = np.maximum(_np_layer_norm(pos, p["ln3_g"], p["ln3_b"]), 0.0) @ p["fc3_w"].T + p["fc3_b"]
    rel = pos[np.asarray(rpi).reshape(-1)].reshape(N, N, NUM_HEADS)  # [q, k, h]
    eb = np.empty((128, 3072), dtype=np.float16)
    for half in range(2):
        for h_l in range(3):
            h = 3 * half + h_l
            e = np.exp(rel[:, :, h].T.astype(np.float32))  # [k, q]
            for kk in range(2):
                off = half * 1536 + h_l * 512 + kk * 256
                eb[:, off:off + 256] = e[kk * 128:(kk + 1) * 128, :]
    return eb


def _build():
    import concourse.mybir as mybir
    import concourse.tile as tile
    from concourse import bacc

    dt = mybir.dt
    nc = bacc.Bacc("TRN2", target_bir_lowering=False, debug=False)
    # [w, {q,k}, c-row (96), half*256 + token] fp16, packed host-side
    qkT_in = nc.declare_dram_parameter("qkT", [WINDOWS_PER_CORE, 2, 96, 512], dt.float16, isOutput=False)
    vext_in = nc.declare_dram_parameter("vext_c", [WINDOWS_PER_CORE, 128, 408], dt.float16, isOutput=False)
    eb_in = nc.declare_dram_parameter("eb", [128, 3072], dt.float16, isOutput=False)
    # raw AV output: [w, q-in-chunk, qc*204 + h*34 + c], c=32 is the denominator
    raw_out = nc.declare_dram_parameter("raw", [WINDOWS_PER_CORE, 128, 408], dt.float32, isOutput=True)

    with tile.TileContext(nc) as tc:
        with (
            tc.tile_pool(name="const", bufs=1) as cp,
            tc.tile_pool(name="io", bufs=4) as io,
            tc.tile_pool(name="vp", bufs=4) as vp,
            tc.tile_pool(name="ptp", bufs=6) as ptp,
            tc.tile_pool(name="ptep", bufs=6) as ptep,
            tc.tile_pool(name="osp", bufs=3) as osp,
            tc.tile_pool(name="ps_s", bufs=2, space="PSUM") as ps_s,
            tc.tile_pool(name="ps_av", bufs=2, space="PSUM") as ps_av,
        ):
            eb = cp.tile([128, 3072], dt.float16, tag="eb")

            def emit_scores(w, half, qt, kt):
                pss = ps_s.tile([128, 1536], dt.float32, tag="scores")
                for h_l in range(3):
                    for kk in range(2):
                        nc.tensor.matmul(
                            pss[:, h_l * 512 + kk * 256: h_l * 512 + (kk + 1) * 256],
                            kt[32 * h_l:32 * h_l + 32,
                               half * 256 + kk * 128: half * 256 + (kk + 1) * 128],
                            qt[32 * h_l:32 * h_l + 32, half * 256:(half + 1) * 256],
                            start=True, stop=True, skip_group_check=True)
                pt = ptp.tile([128, 1536], dt.float16, tag="pt")
                nc.scalar.activation(pt[:], pss[:], mybir.ActivationFunctionType.Exp,
                                     scale=float(SCALE))
                # in-place: P = exp(scale*S) * exp(bias); keeps the WAR for
                # the next window's ACTIVATE on the same tile/semaphore as
                # its data dep so the wait can stay embedded.
                nc.vector.tensor_tensor(
                    out=pt[:], in0=pt[:],
                    in1=eb[:, half * 1536:(half + 1) * 1536],
                    op=mybir.AluOpType.mult)
                return pt

            def emit_av_half(half, pvext, ppte, psa):
                for h_l in range(3):
                    h = 3 * half + h_l
                    for qc in range(2):
                        for kk in range(2):
                            nc.tensor.matmul(
                                psa[:, qc * 204 + h * 34: qc * 204 + (h + 1) * 34],
                                ppte[:, h_l * 512 + kk * 256 + qc * 128:
                                     h_l * 512 + kk * 256 + qc * 128 + 128],
                                pvext[:, kk * 204 + h * 34: kk * 204 + (h + 1) * 34],
                                start=(kk == 0), stop=(kk == 1),
                                skip_group_check=True)

            def emit_out(pw, psa):
                osb = osp.tile([128, 408], dt.float32, tag="osb")
                nc.vector.tensor_copy(osb[:], psa[:, 0:408])
                nc.sync.dma_start(out=raw_out[pw], in_=osb[:])

            hist = []
            for w in range(WINDOWS_PER_CORE + 2):
                prev = hist.pop(0) if w >= 2 else None
                psa = None
                if prev is not None:
                    psa = ps_av.tile([128, 512], dt.float32, tag="av")
                if w < WINDOWS_PER_CORE:
                    # ---- loads: Q^T/K^T [96 c-rows, (half, 256 tok)] fp16 ----
                    qt = io.tile([96, 512], dt.float16, tag="qt")
                    kt = io.tile([96, 512], dt.float16, tag="kt")
                    vext = vp.tile([128, 408], dt.float16, tag="vext")
                    nc.sync.dma_start(out=qt[:], in_=qkT_in[w, 0])
                    nc.sync.dma_start(out=kt[:], in_=qkT_in[w, 1])
                    nc.gpsimd.dma_start(out=vext[:], in_=vext_in[w])
                    if w == 0:
                        nc.gpsimd.dma_start(out=eb[:], in_=eb_in[:])
                    pte0 = emit_scores(w, 0, qt, kt)
                    pte1 = emit_scores(w, 1, qt, kt)
                    if prev is not None:
                        emit_av_half(0, prev[1], prev[2][0], psa)
                        emit_av_half(1, prev[1], prev[2][1], psa)
                        emit_out(prev[0], psa)
                    hist.append((w, vext, (pte0, pte1)))
                elif prev is not None:
                    emit_av_half(0, prev[1], prev[2][0], psa)
                    emit_av_half(1, prev[1], prev[2][1], psa)
                    emit_out(prev[0], psa)
    nc.compile()
    return nc


def _get_nc():
    global _BUILT
    if _BUILT is None:
        _BUILT = _build()
    return _BUILT


def kernel(qkv, H, W, rpi, rpe_biases, pos_proj_w, pos_proj_b, ln1_g, ln1_b,
           fc1_w, fc1_b, ln2_g, ln2_b, fc2_w, fc2_b, ln3_g, ln3_b,
           fc3_w, fc3_b, _trace=False):
    from concourse.bass_utils import run_bass_kernel_spmd

    qkv = np.asarray(qkv, dtype=np.float32)
    params = dict(pos_proj_w=pos_proj_w, pos_proj_b=pos_proj_b, ln1_g=ln1_g,
                  ln1_b=ln1_b, fc1_w=fc1_w, fc1_b=fc1_b, ln2_g=ln2_g,
                  ln2_b=ln2_b, fc2_w=fc2_w, fc2_b=fc2_b, ln3_g=ln3_g,
                  ln3_b=ln3_b, fc3_w=fc3_w, fc3_b=fc3_b)
    params = {k: np.asarray(v, dtype=np.float32) for k, v in params.items()}
    eb = _host_eb(rpi, rpe_biases, params)

    nc = _get_nc()
    in_maps = []
    for c in range(N_CORES):
        b = c // 4
        row0 = (c % 4) * L_PER_CORE
        blk = qkv[:, b, row0:row0 + L_PER_CORE, :]              # [3, 16384, 192]
        win = blk.reshape(3, 8, 8, 8, 32, DIM).transpose(0, 1, 3, 2, 4, 5)
        win = win.reshape(3, WINDOWS_PER_CORE, N, DIM)          # [3, 64, 256, 192]
        qkT = win[0:2].transpose(1, 0, 3, 2).reshape(
            WINDOWS_PER_CORE, 2, 2, 96, N).transpose(0, 1, 3, 2, 4)
        qkT = np.ascontiguousarray(
            qkT.reshape(WINDOWS_PER_CORE, 2, 96, 512)).astype(np.float16)
        v = win[2].reshape(WINDOWS_PER_CORE, 2, 128, 6, 32)
        tmp = np.zeros((WINDOWS_PER_CORE, 2, 128, 6, 34), dtype=np.float16)
        tmp[..., :32] = v
        tmp[..., 32] = 1.0
        vext_c = np.ascontiguousarray(
            tmp.transpose(0, 2, 1, 3, 4).reshape(WINDOWS_PER_CORE, 128, 408))
        in_maps.append({"qkT": qkT, "vext_c": vext_c, "eb": eb})
    res = run_bass_kernel_spmd(nc, in_maps, list(range(N_CORES)), trace=_trace)
    out = np.empty((B, H, W, DIM), dtype=np.float32)
    for c in range(N_CORES):
        b = c // 4
        h0 = (c % 4) * 64
        raw = res.results[c]["raw"].reshape(WINDOWS_PER_CORE, 128, 2, 6, 34)
        o = raw[..., 0:32] / raw[..., 32:33]                 # [w, p, qc, h, d]
        o = o.transpose(0, 2, 1, 3, 4).reshape(WINDOWS_PER_CORE, 256, DIM)
        # windows -> image rows: w = hb*8 + wi, token = i*32 + j
        o = o.reshape(8, 8, 8, 32, DIM).transpose(0, 2, 1, 3, 4)
        out[b, h0:h0 + 64, :, :] = o.reshape(64, W, DIM)
    if _trace:
        return out, res
    return out


# revision 41
# speedup vs baseline: 1.1866x; 1.1866x over previous
"""Trainium2 Bass kernel for windowed attention with dynamic position bias.

Shapes (hardcoded): qkv [3, 2, 65536, 192], H=W=256, window 8x32 (N=256),
6 heads, head_dim 32. 512 windows total, data-parallel over 8 cores
(64 windows each; each core owns a contiguous band of 64 H-rows of one batch).

v8 design:
  - Q^T / K^T built on host (numpy) -> no PE transposes on device.
  - fp16 Q/K/V/P throughout the matmuls; fp32 PSUM accumulation.
  - Scores per half (3 heads) into one [128,1536] PSUM tile; the K=32
    score matmuls use distinct PE row groups (base partitions 0/32/64)
    and run concurrently. One exp ACTIVATE per half (the bottleneck
    engine, ~1.5us/call back-to-back).
  - Position bias applied post-exp as P = exp(scale*S) * exp(bias) with
    exp(bias) precomputed on host; the multiply runs on the Vector engine.
  - AV with ones-column denominator trick, software-pipelined two windows
    behind scores so its inputs are always ready.
  - No on-device softmax division: the [numerators | denominator] PSUM
    block is copied to SBUF and DMA'd out raw; the host does the divide.
"""
import sys
import numpy as np

sys.path.insert(0, "/opt/trn_rl_repo")

H_SP, W_SP = 8, 32
NUM_HEADS = 6
DIM = 192
HEAD_DIM = 32
N = H_SP * W_SP          # 256 tokens per window
LN_EPS = 1e-5
SCALE = HEAD_DIM ** -0.5
B, H, W = 2, 256, 256
L = H * W
N_CORES = 8
WINDOWS_PER_CORE = 64    # 8 hb bands x 8 wi
L_PER_CORE = L // 4      # 16384 tokens (64 H-rows)

_BUILT = None


def _np_layer_norm(x, g, b):
    m = x.mean(axis=-1, keepdims=True)
    v = ((x - m) ** 2).mean(axis=-1, keepdims=True)
    return (x - m) / np.sqrt(v + LN_EPS) * g + b


def _host_eb(rpi, rpe_biases, p):
    """DynamicPosBias MLP + gather -> exp(bias) [128, 3072] fp16.

    col = half*1536 + h_local*512 + kk*256 + q ; partition p = k - kk*128,
    head h = 3*half + h_local. Matches the device score-PSUM layout.
    """
    x = rpe_biases.astype(np.float32)
    pos = x @ p["pos_proj_w"].T + p["pos_proj_b"]
    pos = np.maximum(_np_layer_norm(pos, p["ln1_g"], p["ln1_b"]), 0.0) @ p["fc1_w"].T + p["fc1_b"]
    pos = np.maximum(_np_layer_norm(pos, p["ln2_g"], p["ln2_b"]), 0.0) @ p["fc2_w"].T + p["fc2_b"]
    pos = np.maximum(_np_layer_norm(pos, p["ln3_g"], p["ln3_b"]), 0.0) @ p["fc3_w"].T + p["fc3_b"]
    rel = pos[np.asarray(rpi).reshape(-1)].reshape(N, N, NUM_HEADS)  # [q, k, h]
    eb = np.empty((128, 3072), dtype=np.float16)
    for half in range(2):
        for h_l in range(3):
            h = 3 * half + h_l
            e = np.exp(rel[:, :, h].T.astype(np.float32))  # [k, q]
            for kk in range(2):
                off = half * 1536 + h_l * 512 + kk * 256
                eb[:, off:off + 256] = e[kk * 128:(kk + 1) * 128, :]
    return eb


def _build():
    import concourse.mybir as mybir
    import concourse.tile as tile
    from concourse import bacc

    dt = mybir.dt
    nc = bacc.Bacc("TRN2", target_bir_lowering=False, debug=False)
    # [w, {q,k}, c-row (96), half*256 + token] fp16, packed host-side
    qkT_in = nc.declare_dram_parameter("qkT", [WINDOWS_PER_CORE, 2, 96, 512], dt.float16, isOutput=False)
    vext_in = nc.declare_dram_parameter("vext_c", [WINDOWS_PER_CORE, 128, 408], dt.float16, isOutput=False)
    eb_in = nc.declare_dram_parameter("eb", [128, 3072], dt.float16, isOutput=False)
    # raw AV output: [w, q-in-chunk, qc*204 + h*34 + c], c=32 is the denominator
    raw_out = nc.declare_dram_parameter("raw", [WINDOWS_PER_CORE, 128, 408], dt.float32, isOutput=True)

    with tile.TileContext(nc) as tc:
        with (
            tc.tile_pool(name="const", bufs=1) as cp,
            tc.tile_pool(name="io", bufs=4) as io,
            tc.tile_pool(name="vp", bufs=4) as vp,
            tc.tile_pool(name="ptp", bufs=6) as ptp,
            tc.tile_pool(name="ptep", bufs=6) as ptep,
            tc.tile_pool(name="osp", bufs=3) as osp,
            tc.tile_pool(name="ps_s", bufs=2, space="PSUM") as ps_s,
            tc.tile_pool(name="ps_av", bufs=2, space="PSUM") as ps_av,
        ):
            eb = cp.tile([128, 3072], dt.float16, tag="eb")

            def emit_scores(w, half, qt, kt):
                pss = ps_s.tile([128, 1536], dt.float32, tag="scores")
                for h_l in range(3):
                    for kk in range(2):
                        nc.tensor.matmul(
                            pss[:, h_l * 512 + kk * 256: h_l * 512 + (kk + 1) * 256],
                            kt[32 * h_l:32 * h_l + 32,
                               half * 256 + kk * 128: half * 256 + (kk + 1) * 128],
                            qt[32 * h_l:32 * h_l + 32, half * 256:(half + 1) * 256],
                            start=True, stop=True, skip_group_check=True)
                pt = ptp.tile([128, 1536], dt.float16, tag="pt")
                nc.scalar.activation(pt[:], pss[:], mybir.ActivationFunctionType.Exp,
                                     scale=float(SCALE))
                # in-place: P = exp(scale*S) * exp(bias); keeps the WAR for
                # the next window's ACTIVATE on the same tile/semaphore as
                # its data dep so the wait can stay embedded.
                nc.vector.tensor_tensor(
                    out=pt[:], in0=pt[:],
                    in1=eb[:, half * 1536:(half + 1) * 1536],
                    op=mybir.AluOpType.mult)
                return pt

            def emit_av_half(half, pvext, ppte, psa):
                for h_l in range(3):
                    h = 3 * half + h_l
                    for qc in range(2):
                        for kk in range(2):
                            nc.tensor.matmul(
                                psa[:, qc * 204 + h * 34: qc * 204 + (h + 1) * 34],
                                ppte[:, h_l * 512 + kk * 256 + qc * 128:
                                     h_l * 512 + kk * 256 + qc * 128 + 128],
                                pvext[:, kk * 204 + h * 34: kk * 204 + (h + 1) * 34],
                                start=(kk == 0), stop=(kk == 1),
                                skip_group_check=True)

            def emit_out(pw, psa):
                osb = osp.tile([128, 408], dt.float32, tag="osb")
                nc.vector.tensor_copy(osb[:], psa[:, 0:408])
                nc.sync.dma_start(out=raw_out[pw], in_=osb[:])

            hist = []
            for w in range(WINDOWS_PER_CORE + 2):
                prev = hist.pop(0) if w >= 2 else None
                psa = None
                if prev is not None:
                    psa = ps_av.tile([128, 512], dt.float32, tag="av")
                if w < WINDOWS_PER_CORE:
                    # ---- loads: Q^T/K^T [96 c-rows, (half, 256 tok)] fp16 ----
                    qt = io.tile([96, 512], dt.float16, tag="qt")
                    kt = io.tile([96, 512], dt.float16, tag="kt")
                    vext = vp.tile([128, 408], dt.float16, tag="vext")
                    nc.sync.dma_start(out=qt[:], in_=qkT_in[w, 0])
                    nc.gpsimd.dma_start(out=kt[:], in_=qkT_in[w, 1])
                    nc.gpsimd.dma_start(out=vext[:], in_=vext_in[w])
                    if w == 0:
                        nc.gpsimd.dma_start(out=eb[:], in_=eb_in[:])
                    pte0 = emit_scores(w, 0, qt, kt)
                    pte1 = emit_scores(w, 1, qt, kt)
                    if prev is not None:
                        emit_av_half(0, prev[1], prev[2][0], psa)
                        emit_av_half(1, prev[1], prev[2][1], psa)
                        emit_out(prev[0], psa)
                    hist.append((w, vext, (pte0, pte1)))
                elif prev is not None:
                    emit_av_half(0, prev[1], prev[2][0], psa)
                    emit_av_half(1, prev[1], prev[2][1], psa)
                    emit_out(prev[0], psa)
    nc.compile()
    return nc


def _get_nc():
    global _BUILT
    if _BUILT is None:
        _BUILT = _build()
    return _BUILT


def kernel(qkv, H, W, rpi, rpe_biases, pos_proj_w, pos_proj_b, ln1_g, ln1_b,
           fc1_w, fc1_b, ln2_g, ln2_b, fc2_w, fc2_b, ln3_g, ln3_b,
           fc3_w, fc3_b, _trace=False):
    from concourse.bass_utils import run_bass_kernel_spmd

    qkv = np.asarray(qkv, dtype=np.float32)
    params = dict(pos_proj_w=pos_proj_w, pos_proj_b=pos_proj_b, ln1_g=ln1_g,
                  ln1_b=ln1_b, fc1_w=fc1_w, fc1_b=fc1_b, ln2_g=ln2_g,
                  ln2_b=ln2_b, fc2_w=fc2_w, fc2_b=fc2_b, ln3_g=ln3_g,
                  ln3_b=ln3_b, fc3_w=fc3_w, fc3_b=fc3_b)
    params = {k: np.asarray(v, dtype=np.float32) for k, v in params.items()}
    eb = _host_eb(rpi, rpe_biases, params)

    nc = _get_nc()
    in_maps = []
    for c in range(N_CORES):
        b = c // 4
        row0 = (c % 4) * L_PER_CORE
        blk = qkv[:, b, row0:row0 + L_PER_CORE, :]              # [3, 16384, 192]
        win = blk.reshape(3, 8, 8, 8, 32, DIM).transpose(0, 1, 3, 2, 4, 5)
        win = win.reshape(3, WINDOWS_PER_CORE, N, DIM)          # [3, 64, 256, 192]
        qkT = win[0:2].transpose(1, 0, 3, 2).reshape(
            WINDOWS_PER_CORE, 2, 2, 96, N).transpose(0, 1, 3, 2, 4)
        qkT = np.ascontiguousarray(
            qkT.reshape(WINDOWS_PER_CORE, 2, 96, 512)).astype(np.float16)
        v = win[2].reshape(WINDOWS_PER_CORE, 2, 128, 6, 32)
        tmp = np.zeros((WINDOWS_PER_CORE, 2, 128, 6, 34), dtype=np.float16)
        tmp[..., :32] = v
        tmp[..., 32] = 1.0
        vext_c = np.ascontiguousarray(
            tmp.transpose(0, 2, 1, 3, 4).reshape(WINDOWS_PER_CORE, 128, 408))
        in_maps.append({"qkT": qkT, "vext_c": vext_c, "eb": eb})
    res = run_bass_kernel_spmd(nc, in_maps, list(range(N_CORES)), trace=_trace)
    out = np.empty((B, H, W, DIM), dtype=np.float32)
    for c in range(N_CORES):
        b = c // 4
        h0 = (c % 4) * 64
        raw = res.results[c]["raw"].reshape(WINDOWS_PER_CORE, 128, 2, 6, 34)
        o = raw[..., 0:32] / raw[..., 32:33]                 # [w, p, qc, h, d]
        o = o.transpose(0, 2, 1, 3, 4).reshape(WINDOWS_PER_CORE, 256, DIM)
        # windows -> image rows: w = hb*8 + wi, token = i*32 + j
        o = o.reshape(8, 8, 8, 32, DIM).transpose(0, 2, 1, 3, 4)
        out[b, h0:h0 + 64, :, :] = o.reshape(64, W, DIM)
    if _trace:
        return out, res
    return out
